# revision 1
# baseline (speedup 1.0000x reference)
"""Trainium2 Bass kernel: CrossAttentionFusion (dense transformer block pair).

Math notes (vs the reference):
  - seq_len-1 cross attention: softmax over a single key == 1, so
    mha1(q_in, kv_in) == kv_in @ (Wo@Wv).T + (Wo@bv + bo).  q/k projections are
    dead code; the two projections fuse into ONE 768x768 matmul (host-fused).
  - Transposed layout: activations live as [feature, batch]; matmuls are
    lhsT(=W.T, stationary) x rhs(=x.T, moving) -> out = (x@W.T).T.
    LayerNorm reduces over features (= partitions) with a ones-vector matmul on
    the PE; per-sample stats are broadcast back over partitions with K=1 ones
    matmuls.
  - Data-parallel over batch: 16384 rows -> 8 cores x 2048.
  - Mixed precision matmuls: attention + fused-projection in bf16; the FFN2
    matmuls (h @ W2.T, 40%% of FLOPs) in fp8e4 (TRN e4m3) with
    perf_mode=DoubleRow (2 fp8 weights per PE cell -> ~2x measured PE
    throughput at FD=512).  fp8 weights are scaled x32 into e4m3's normal
    range (descaled in the following DVE op); gelu writes h directly as fp8
    (single rounding).  FFN2's bias b2 is folded into downstream biases on
    the host (img2/txt2 are only consumed by linear ops), freeing the DVE
    epilogue for the 1/32 descale.  Measured rel err 0.0145 vs the 0.02 gate.
  - One flat 12-stage software pipeline (8 block stages + 4 fused-projection
    stages) with 1-stage skew: each stage's LN broadcast/apply and FFN are
    emitted inside the NEXT stage's attention matmuls, including across phase
    boundaries, so the PE never waits on an LN row-stat chain.
  - FFN weights for the second branch reuse the first branch's SBUF buffers
    (tag-reuse WAR); all other weights are prefetched at t=0.
"""

import numpy as np
import ml_dtypes

import concourse.bass as bass
from concourse import bacc, tile, mybir
from concourse.bass_utils import run_bass_kernel_spmd

BF16 = ml_dtypes.bfloat16
FP8 = ml_dtypes.float8_e4m3  # TRN fp8_e4m3: max normal 240
DT_BF = mybir.dt.bfloat16
DT_F32 = mybir.dt.float32
DT_F8 = mybir.dt.float8e4
AF = mybir.ActivationFunctionType
ALU = mybir.AluOpType
DR = mybir.MatmulPerfMode.DoubleRow

B_FULL, E, H = 16384, 768, 8
F = 4 * E  # 3072
N_CORES = 8
BS = B_FULL // N_CORES  # 2048
EPS = 1e-5
P = 128
KE = E // P  # 6
KF = F // P  # 24
WS = 32.0  # fp8 weight scale


def build(bs=BS, strip=512, nk1=0, nk2=24, use_gelu=True, num_devices=N_CORES,
          sim_safe=False):
    """Emit the per-core Bass program (SPMD: same program on every core)."""
    nstrip = bs // strip
    assert nstrip * strip == bs
    nb1 = KE - nk1  # bf16 k-tiles in FFN1
    nb2 = KF - nk2  # bf16 k-tiles in FFN2
    assert nk1 % 2 == 0 and nk2 % 2 == 0
    w1s = WS if nk1 else 1.0

    nc = bacc.Bacc(
        "TRN2", target_bir_lowering=False, debug=False, num_devices=num_devices
    )

    # ---- DRAM I/O ----
    d_img = nc.dram_tensor("imageT", [E, bs], DT_BF, kind="ExternalInput")
    d_txt = nc.dram_tensor("textT", [E, bs], DT_BF, kind="ExternalInput")
    d_watt = {
        "it": nc.dram_tensor("watt_it", [E, E], DT_BF, kind="ExternalInput"),
        "ti": nc.dram_tensor("watt_ti", [E, E], DT_BF, kind="ExternalInput"),
    }
    d_wfp = nc.dram_tensor("wfp", [2 * E, E], DT_BF, kind="ExternalInput")
    d_w1b, d_w1q, d_w2b, d_w2q = {}, {}, {}, {}
    for p in ("fi", "ft"):
        if nb1:
            d_w1b[p] = nc.dram_tensor(f"w1b_{p}", [nb1 * P, F], DT_BF,
                                      kind="ExternalInput")
        if nk1:
            d_w1q[p] = nc.dram_tensor(f"w1q_{p}", [nk1 * P, F], DT_F8,
                                      kind="ExternalInput")
        if nb2:
            d_w2b[p] = nc.dram_tensor(f"w2b_{p}", [nb2 * P, E], DT_BF,
                                      kind="ExternalInput")
        if nk2:
            d_w2q[p] = nc.dram_tensor(f"w2q_{p}", [nk2 * P, E], DT_F8,
                                      kind="ExternalInput")
    bias_specs = {
        "batt_it": KE, "g_img": KE, "b_img": KE, "b1_fi": KF,
        "batt_ti": KE, "g_txt": KE, "b_txt": KE, "b1_ft": KF,
        "bfp": KE, "g_fp": KE, "b_fp_ln": KE,
    }
    d_bias = {
        n: nc.dram_tensor(n, [P, k], DT_F32, kind="ExternalInput")
        for n, k in bias_specs.items()
    }
    d_out = nc.dram_tensor("outT", [E, bs], DT_F32, kind="ExternalOutput")

    def dview(d, kt=KE):  # [kt*P, n] dram -> [p, kt, n] view
        return d.ap().rearrange("(kt p) n -> p kt n", p=P)

    with tile.TileContext(nc) as tc:
        from contextlib import ExitStack

        with ExitStack() as ctx:
            const = ctx.enter_context(tc.tile_pool(name="const", bufs=1))
            wp = ctx.enter_context(tc.tile_pool(name="wp", bufs=1))
            pin = ctx.enter_context(tc.tile_pool(name="pin", bufs=3))
            pwork = ctx.enter_context(tc.tile_pool(name="pwork", bufs=2))
            ph = ctx.enter_context(tc.tile_pool(name="ph", bufs=1))
            prow = ctx.enter_context(tc.tile_pool(name="prow", bufs=1))
            pst = ctx.enter_context(tc.tile_pool(name="pst", bufs=3))
            pps = ctx.enter_context(
                tc.tile_pool(name="pps", bufs=2, space=bass.MemorySpace.PSUM)
            )
            p3 = ctx.enter_context(tc.tile_pool(name="p3", bufs=3))
            pdram = ctx.enter_context(
                tc.tile_pool(name="pdram", bufs=1, space=bass.MemorySpace.DRAM)
            )

            # ---- startup-critical DMAs: watt_it + first strip, k-interleaved
            ones_sb = const.tile([P, P], DT_BF)
            nc.vector.memset(ones_sb[:], 1.0)
            eps_sb = const.tile([1, 1], DT_F32)
            nc.vector.memset(eps_sb[:], EPS)
            watt_sb = {
                pfx: const.tile([P, KE, E], DT_BF, tag=f"watt_{pfx}",
                                name=f"watt_{pfx}")
                for pfx in ("it", "ti")
            }
            rhs0 = pin.tile([P, KE, strip], DT_BF, tag="rhs_in", name="in_rhs0")
            res0 = pin.tile([P, KE, strip], DT_BF, tag="res_in", name="in_res0")
            for k in range(KE):
                nc.sync.dma_start(watt_sb["it"][:, k, :],
                                  dview(d_watt["it"])[:, k, :])
                nc.sync.dma_start(rhs0[:, k, :], dview(d_txt)[:, k, 0:strip])
            for k in range(KE):
                nc.sync.dma_start(res0[:, k, :], dview(d_img)[:, k, 0:strip])
            bias_sb = {}
            for n, k in bias_specs.items():
                t = const.tile([P, k], DT_F32, tag=f"bias_{n}")
                nc.sync.dma_start(t[:], d_bias[n].ap())
                bias_sb[n] = t

            # ---- bulk weight prefetch (fi branch + watt_ti + wfp) ----
            def load_ffn_w(pfx):
                """(Re)allocate the FFN weight tiles and start their DMAs.
                Same tags + bufs=1 -> second branch reuses the buffers with
                auto WAR deps on the first branch's matmuls."""
                w = {}
                if nb1:
                    w["w1b"] = wp.tile([P, nb1, F], DT_BF, tag="w1b",
                                       name=f"w1b_{pfx}")
                    for k in range(nb1):
                        nc.sync.dma_start(w["w1b"][:, k, :],
                                          dview(d_w1b[pfx], nb1)[:, k, :])
                if nk1:
                    w["w1q"] = wp.tile([P, nk1, F], DT_F8, tag="w1q",
                                       name=f"w1q_{pfx}")
                    for k in range(nk1):
                        nc.sync.dma_start(w["w1q"][:, k, :],
                                          dview(d_w1q[pfx], nk1)[:, k, :])
                if nb2:
                    w["w2b"] = wp.tile([P, nb2, E], DT_BF, tag="w2b",
                                       name=f"w2b_{pfx}")
                    for k in range(nb2):
                        nc.sync.dma_start(w["w2b"][:, k, :],
                                          dview(d_w2b[pfx], nb2)[:, k, :])
                if nk2:
                    w["w2q"] = wp.tile([P, nk2, E], DT_F8, tag="w2q",
                                       name=f"w2q_{pfx}")
                    for k in range(0, nk2, 4):
                        nc.sync.dma_start(
                            w["w2q"][:, k:k + 4, :],
                            dview(d_w2q[pfx], nk2)[:, k:k + 4, :])
                return w

            w_fi = load_ffn_w("fi")
            for k in range(KE):
                nc.sync.dma_start(watt_sb["ti"][:, k, :],
                                  dview(d_watt["ti"])[:, k, :])
            wfp_sb = const.tile([P, 2 * KE, E], DT_BF)
            for k in range(2 * KE):
                nc.sync.dma_start(wfp_sb[:, k, :], dview(d_wfp, 2 * KE)[:, k, :])

            # ---- internal DRAM trunk ----
            d_img2 = [
                pdram.tile([P, KE, strip], DT_BF, tag=f"img2_{s}",
                           name=f"img2_{s}") for s in range(nstrip)
            ]
            d_txt2 = [
                pdram.tile([P, KE, strip], DT_BF, tag=f"txt2_{s}",
                           name=f"txt2_{s}") for s in range(nstrip)
            ]

            # ---------- helpers ----------
            def load_strip_ext(dsrc, sl, tag, tile_=None):
                t = tile_ or pin.tile([P, KE, strip], DT_BF, tag=tag,
                                      name=f"in_{tag}")
                for k in range(KE):
                    nc.sync.dma_start(t[:, k, :], dsrc[:, k, sl])
                return t

            def load_strip_trunk(dtile, tag):
                t = pin.tile([P, KE, strip], DT_BF, tag=tag, name=f"in_{tag}")
                nc.sync.dma_start(t[:], dtile[:])
                return t

            def dense_att(rhs_t, resid_t, w_sb, b_sb, hooks):
                """r[m] = (x @ Wc.T).T[m] + b[m] + resid[m]  (bf16 out).
                hooks: dict m -> callable emitted after m's epilogue."""
                r = pwork.tile([P, KE, strip], DT_BF, tag="r1", name="r1")
                for m in range(KE):
                    ps = pps.tile([P, strip], DT_F32, tag="mm", bufs=4, name="ps")
                    for k in range(KE):
                        nc.tensor.matmul(
                            ps[:],
                            w_sb[:, k, m * P:(m + 1) * P],
                            rhs_t[:, k, :],
                            start=(k == 0),
                            stop=(k == KE - 1),
                        )
                    nc.vector.scalar_tensor_tensor(
                        r[:, m, :], ps[:], b_sb[:, m:m + 1], resid_t[:, m, :],
                        ALU.add, ALU.add,
                    )
                    if m in hooks:
                        hooks[m]()
                return r

            def ln_presum(r):
                """DVE feature pre-sums of r and r^2 -> [P,strip] bf16 pair."""
                s = pwork.tile([P, strip], DT_BF, tag="s", name="s")
                nc.vector.tensor_tensor(s[:], r[:, 0, :], r[:, 1, :], ALU.add)
                for k in range(2, KE):
                    nc.vector.tensor_tensor(s[:], s[:], r[:, k, :], ALU.add)
                sq = pwork.tile([P, strip], DT_BF, tag="sq", name="sq")
                tmp = pwork.tile([P, strip], DT_BF, tag="sqtmp", name="sqtmp")
                nc.vector.tensor_tensor(sq[:], r[:, 0, :], r[:, 0, :], ALU.mult)
                for k in range(1, KE):
                    nc.vector.tensor_tensor(tmp[:], r[:, k, :], r[:, k, :],
                                            ALU.mult)
                    nc.vector.tensor_tensor(sq[:], sq[:], tmp[:], ALU.add)
                return s, sq

            def ln_redrows(ssq):
                """PE partition-reduce + row-stat chain -> (mean, rstd) rows."""
                s, sq = ssq
                red0 = pps.tile([1, strip], DT_F32, tag="hps", bufs=2,
                                name="red0")
                red1 = pps.tile([1, strip], DT_F32, tag="ops", bufs=2,
                                name="red1")
                nc.tensor.matmul(red0[:], ones_sb[:, 0:1], s[:], start=True,
                                 stop=True)
                nc.tensor.matmul(red1[:], ones_sb[:, 0:1], sq[:], start=True,
                                 stop=True)
                mean_bf = prow.tile([1, strip], DT_BF, tag="mean", name="mean")
                nc.scalar.activation(mean_bf[:], red0[:], AF.Copy, scale=1.0 / E)
                msq = prow.tile([1, strip], DT_F32, tag="msq", name="msq")
                nc.vector.tensor_tensor(msq[:], mean_bf[:], mean_bf[:], ALU.mult)
                var = prow.tile([1, strip], DT_F32, tag="var", name="var")
                nc.vector.scalar_tensor_tensor(
                    var[:], red1[:], 1.0 / E, msq[:], ALU.mult, ALU.subtract
                )
                rstd_bf = prow.tile([1, strip], DT_BF, tag="rstdbf",
                                    name="rstdbf")
                if sim_safe:
                    std = prow.tile([1, strip], DT_F32, tag="std", name="std")
                    nc.scalar.activation(std[:], var[:], AF.Sqrt,
                                         bias=eps_sb[0:1, 0:1])
                    rstd = prow.tile([1, strip], DT_F32, tag="rstd", name="rstd")
                    nc.vector.reciprocal(rstd[:], std[:])
                    nc.vector.tensor_copy(rstd_bf[:], rstd[:])
                else:
                    nc.scalar.activation(
                        rstd_bf[:], var[:], AF.Abs_reciprocal_sqrt,
                        bias=eps_sb[0:1, 0:1],
                    )
                return mean_bf, rstd_bf

            def ln_bcast_apply(r, rows, out_emit):
                """PE K=1 broadcast of stats over partitions + DVE/ACT apply."""
                mean_bf, rstd_bf = rows
                mb = pps.tile([P, strip], DT_F32, tag="hps", bufs=2, name="mb")
                nc.tensor.matmul(mb[:], ones_sb[0:1, :], mean_bf[:], start=True,
                                 stop=True)
                rb = pps.tile([P, strip], DT_F32, tag="ops", bufs=2, name="rb")
                nc.tensor.matmul(rb[:], ones_sb[0:1, :], rstd_bf[:], start=True,
                                 stop=True)
                for k in range(KE):
                    t = pwork.tile([P, strip], DT_BF, tag="lnt", name="lnt")
                    nc.vector.tensor_tensor(t[:], r[:, k, :], mb[:],
                                            ALU.subtract)
                    nc.vector.tensor_tensor(t[:], t[:], rb[:], ALU.mult)
                    out_emit(k, t)

            def ln_to_x(r, rows, g_sb, b_sb):
                """LN apply -> x (bf16 tiles) + fp8 copies of the nk1 tail
                k-tiles (FFN1 DoubleRow operands)."""
                x = [
                    pwork.tile([P, strip], DT_BF, tag=f"xk{k}", name=f"x{k}")
                    for k in range(KE)
                ]
                xq = (pwork.tile([P, nk1, strip], DT_F8, tag="xq", name="xq")
                      if nk1 else None)

                def emit(k, t):
                    nc.scalar.activation(
                        x[k][:], t[:], AF.Identity,
                        bias=b_sb[:, k:k + 1], scale=g_sb[:, k:k + 1],
                    )
                    if nk1 and k >= nb1:
                        nc.scalar.activation(
                            xq[:, k - nb1, :], t[:], AF.Identity,
                            bias=b_sb[:, k:k + 1], scale=g_sb[:, k:k + 1],
                        )

                ln_bcast_apply(r, rows, emit)
                return x, xq

            def ffn(xxq, w, b1_sb, dtile):
                """dtile[:, m, :] = x + (gelu(x@W1.T+b1))@W2.T   (b2 folded).

                W2 (and W1's fp8 part) are x32-scaled; descale via ACT scale
                and the DVE stt scalar."""
                x, xq = xxq
                hb = (ph.tile([P, nb2, strip], DT_BF, tag="hb", name="hb")
                      if nb2 else None)
                hq = (ph.tile([P, nk2, strip], DT_F8, tag="hq", name="hq")
                      if nk2 else None)
                for m in range(KF):
                    hps = pps.tile([P, strip], DT_F32, tag="hps", bufs=2,
                                   name="hps")
                    for k in range(nb1):
                        nc.tensor.matmul(
                            hps[:], w["w1b"][:, k, m * P:(m + 1) * P], x[k][:],
                            start=(k == 0), stop=(nk1 == 0 and k == nb1 - 1),
                        )
                    for j in range(0, nk1, 2):
                        nc.tensor.matmul(
                            hps[:], w["w1q"][:, j:j + 2, m * P:(m + 1) * P],
                            xq[:, j:j + 2, :],
                            start=(nb1 == 0 and j == 0), stop=(j == nk1 - 2),
                            perf_mode=DR,
                        )
                    ho = hq[:, m - nb2, :] if m >= nb2 else hb[:, m, :]
                    nc.scalar.activation(
                        ho, hps[:],
                        AF.Gelu if use_gelu else AF.Identity,
                        bias=b1_sb[:, m:m + 1], scale=1.0 / w1s,
                    )
                for m in range(KE):
                    ops = pps.tile([P, strip], DT_F32, tag="ops", bufs=2,
                                   name="ops")
                    for k in range(nb2):
                        nc.tensor.matmul(
                            ops[:], w["w2b"][:, k, m * P:(m + 1) * P],
                            hb[:, k, :],
                            start=(k == 0), stop=(nk2 == 0 and k == nb2 - 1),
                        )
                    for j in range(0, nk2, 2):
                        nc.tensor.matmul(
                            ops[:], w["w2q"][:, j:j + 2, m * P:(m + 1) * P],
                            hq[:, j:j + 2, :],
                            start=(nb2 == 0 and j == 0), stop=(j == nk2 - 2),
                            perf_mode=DR,
                        )
                    st = pst.tile([P, strip], DT_BF, tag="stg", name="stg")
                    nc.vector.scalar_tensor_tensor(
                        st[:], ops[:], 1.0 / WS, x[m][:], ALU.mult, ALU.add,
                    )
                    nc.sync.dma_start(dtile[:, m, :], st[:])

            # ---------- the 12-stage flat pipeline ----------
            pend = [None]  # block-stage pend awaiting ln_to_x + ffn
            fpend = [None]  # fp-stage pend awaiting redrows / fp_finish

            def emit_block_payload_ln():
                pd = pend[0]
                if pd is None:
                    return
                pd["x"] = ln_to_x(pd["r"], pd["rows"], pd["g"], pd["b"])

            def emit_block_payload_ffn():
                pd = pend[0]
                if pd is None:
                    return
                ffn(pd["x"], pd["w"](), pd["b1"], pd["dout"])
                pend[0] = None

            def block_stage(i):
                pfx = "fi" if i < nstrip else "ft"
                s = i % nstrip
                sl = slice(s * strip, (s + 1) * strip)
                if pfx == "fi":
                    rhs_t = (rhs0 if i == 0 else
                             load_strip_ext(dview(d_txt), sl, "rhs_in"))
                    res_t = (res0 if i == 0 else
                             load_strip_ext(dview(d_img), sl, "res_in"))
                    watt, batt = watt_sb["it"], bias_sb["batt_it"]
                    g_ln, b_ln = bias_sb["g_img"], bias_sb["b_img"]
                    b1, dout, w = bias_sb["b1_fi"], d_img2, (lambda: w_fi)
                else:
                    rhs_t = load_strip_trunk(d_img2[s], "rhs_in")
                    res_t = load_strip_ext(dview(d_txt), sl, "res_in")
                    watt, batt = watt_sb["ti"], bias_sb["batt_ti"]
                    g_ln, b_ln = bias_sb["g_txt"], bias_sb["b_txt"]
                    b1, dout, w = bias_sb["b1_ft"], d_txt2, (lambda: w_ft[0])

                hooks = {}
                if i == 1:
                    hooks[1] = lambda: pend[0].update(
                        rows=ln_redrows(pend[0]["ssq"]))
                if pend[0] is not None:
                    hooks[3] = emit_block_payload_ln

                r = dense_att(rhs_t, res_t, watt, batt, hooks)
                ssq = ln_presum(r)
                emit_block_payload_ffn()
                if i == 4:
                    # second branch's FFN weights into the same buffers.
                    # Must come AFTER stage 3's ffn emission (just above) so
                    # the tag-reuse WAR deps cover all fi-weight readers.
                    w_ft[0] = load_ffn_w("ft")
                rows = None if i == 0 else ln_redrows(ssq)
                pend[0] = dict(r=r, rows=rows, ssq=ssq, g=g_ln, b=b_ln,
                               w=w, b1=b1, dout=dout[s])

            def fp_finish(pd):
                def emit_out(k, t):
                    o = p3.tile([P, strip], DT_F32, tag="of32", name="of32")
                    nc.scalar.activation(
                        o[:], t[:],
                        AF.Gelu if use_gelu else AF.Identity,
                        bias=bias_sb["b_fp_ln"][:, k:k + 1],
                        scale=bias_sb["g_fp"][:, k:k + 1],
                    )
                    nc.sync.dma_start(outv[:, k, pd["sl"]], o[:])

                ln_bcast_apply(pd["r3"], pd["rows"], emit_out)

            outv = dview(d_out)

            def fp_stage(s):
                sl = slice(s * strip, (s + 1) * strip)
                a_in = load_strip_trunk(d_img2[s], "rhs_in")
                b_in = load_strip_trunk(d_txt2[s], "res_in")
                r3 = pwork.tile([P, KE, strip], DT_BF, tag="r1", name="r3")
                for m in range(KE):
                    zps = pps.tile([P, strip], DT_F32, tag="mm", bufs=4,
                                   name="zps")
                    for k in range(2 * KE):
                        src = a_in if k < KE else b_in
                        nc.tensor.matmul(
                            zps[:], wfp_sb[:, k, m * P:(m + 1) * P],
                            src[:, k % KE, :],
                            start=(k == 0), stop=(k == 2 * KE - 1),
                        )
                    nc.scalar.activation(
                        r3[:, m, :], zps[:], AF.Identity,
                        bias=bias_sb["bfp"][:, m:m + 1],
                    )
                    if m == 1 and fpend[0] is not None and \
                            "rows" not in fpend[0]:
                        fpend[0]["rows"] = ln_redrows(fpend[0]["ssq"])
                    if m == 3 and pend[0] is not None:
                        emit_block_payload_ln()
                    if m == 5 and fpend[0] is not None and \
                            "rows" in fpend[0]:
                        fp_finish(fpend[0])
                        fpend[0] = None
                ssq3 = ln_presum(r3)
                if pend[0] is not None:
                    emit_block_payload_ffn()
                fpend[0] = dict(r3=r3, sl=sl, ssq=ssq3)

            w_ft = [None]
            for i in range(2 * nstrip):
                block_stage(i)
            for s in range(nstrip):
                fp_stage(s)
            # tail: last fp stage's LN + gelu + out DMA
            fpend[0]["rows"] = ln_redrows(fpend[0]["ssq"])
            fp_finish(fpend[0])

    nc.compile()
    return nc


# ---------------- host side ----------------

_BUILT = {}


def _get_nc(key):
    if key not in _BUILT:
        _BUILT[key] = build(*key)
    return _BUILT[key]


def _packv(v, ktiles):
    return np.ascontiguousarray(np.asarray(v, np.float32).reshape(ktiles, P).T)


def prep_inputs(inputs, bs=BS, n_cores=N_CORES, nk1=0, nk2=24):
    f32 = np.float32
    nb1 = KE - nk1
    nb2 = KF - nk2
    w1s = WS if nk1 else 1.0
    g = lambda n: np.asarray(inputs[n], f32)
    common = {}
    wc = {}
    for pfx in ("it", "ti"):
        wc[pfx] = g(f"{pfx}_Wo") @ g(f"{pfx}_Wv")
        common[f"watt_{pfx}"] = np.ascontiguousarray(wc[pfx].T).astype(BF16)
    # fold FFN2 bias b2 into downstream consumers of img2/txt2 (all linear):
    #   att2 kv = img2 + b2_fi  ->  batt_ti += Wc_ti @ b2_fi
    #   fused proj: bfp += fp_W[:, :E] @ b2_fi + fp_W[:, E:] @ b2_ft
    batt_it = g("it_Wo") @ g("it_bv") + g("it_bo")
    batt_ti = (g("ti_Wo") @ g("ti_bv") + g("ti_bo") + wc["ti"] @ g("fi_b2"))
    bfp = (g("fp_b") + g("fp_W")[:, :E] @ g("fi_b2")
           + g("fp_W")[:, E:] @ g("ft_b2"))
    common["batt_it"] = _packv(batt_it, KE)
    common["batt_ti"] = _packv(batt_ti, KE)
    common["bfp"] = _packv(bfp, KE)
    for p in ("fi", "ft"):
        w1t = np.ascontiguousarray(g(f"{p}_W1").T) * w1s  # [E, F]
        if nb1:
            common[f"w1b_{p}"] = w1t[: nb1 * P].astype(BF16)
        if nk1:
            common[f"w1q_{p}"] = np.clip(w1t[nb1 * P:], -240, 240).astype(FP8)
        w2t = np.ascontiguousarray(g(f"{p}_W2").T) * WS  # [F, E]
        if nb2:
            common[f"w2b_{p}"] = w2t[: nb2 * P].astype(BF16)
        if nk2:
            common[f"w2q_{p}"] = np.clip(w2t[nb2 * P:], -240, 240).astype(FP8)
    common["wfp"] = np.ascontiguousarray(g("fp_W").T).astype(BF16)
    common["b1_fi"] = _packv(g("fi_b1"), KF)
    common["b1_ft"] = _packv(g("ft_b1"), KF)
    common["g_img"] = _packv(g("ln_img_g"), KE)
    common["b_img"] = _packv(g("ln_img_b"), KE)
    common["g_txt"] = _packv(g("ln_text_g"), KE)
    common["b_txt"] = _packv(g("ln_text_b"), KE)
    common["g_fp"] = _packv(g("fp_ln_g"), KE)
    common["b_fp_ln"] = _packv(g("fp_ln_b"), KE)

    imgT = g("image_embed").T.astype(BF16)
    txtT = g("text_embed").T.astype(BF16)
    in_maps = []
    for c in range(n_cores):
        sl = slice(c * bs, (c + 1) * bs)
        m = dict(common)
        m["imageT"] = np.ascontiguousarray(imgT[:, sl])
        m["textT"] = np.ascontiguousarray(txtT[:, sl])
        in_maps.append(m)
    return in_maps


NK1, NK2 = 0, 24
CFG = (BS, 512, NK1, NK2, True, N_CORES)


def kernel(**inputs):
    nc = _get_nc(CFG)
    in_maps = prep_inputs(inputs, nk1=NK1, nk2=NK2)
    res = run_bass_kernel_spmd(nc, in_maps, core_ids=list(range(N_CORES)))
    out = np.concatenate(
        [res.results[c]["outT"] for c in range(N_CORES)], axis=1
    )  # [E, B]
    return np.ascontiguousarray(out.T).astype(np.float32)

